# revision 58
# baseline (speedup 1.0000x reference)
"""GNN attention (GAT-style single-target-node) kernel for 8 Trainium2 cores.

Problem:  x [32, 50000, 64], a [128, 1], node_index scalar, adj_mask [50000]
  tgt_score = x[:, idx] @ a[:64]                             # [B]
  e = leaky_relu(tgt_score[:, None] + x @ a[64:], 0.01)      # [B, N]
  attention = softmax(where(adj>0, e, -9e15), axis=1) * adj  # [B, N]

Key observation: rows with adj_mask == 0 contribute exp(-9e15) = 0 to the
softmax denominator and their output is *exactly* 0 (softmax * adj).  So the
device only ever needs x at the ~25k unmasked nodes: the host compacts
x[:, keep, :] (fp16) before upload and scatters the compact attention back
into a zero [B, N] canvas afterwards.  This halves HBM traffic vs the dense
kernel and removes the mask-bias grid entirely.

Sharding: data-parallel over batch (32 = 8 cores x 4 batches/core), the 4
batches paired into 2 batch-pairs.  Each core computes complete softmax rows,
so no cross-core reductions.

Per pair the host lays compact x out as xh[pair, bi*64+d, col] (fp16, both
batches of a pair stacked on the contraction axis).  Each 128-node chunk is
the *stationary* matmul operand [K=128, M=128 nodes] and the tiny constant
a-matrix [128, 2] streams as the moving operand, so LDWEIGHTS itself is the
data pass (128 values/cycle) and out = [128 nodes, 2 batches] lands
interleaved in PSUM (one bank per 256 chunks; a single bank for the seed-0
mask, where ch = 195 <= 256).  Column permutation
col = c*128 + p <-> compact slot p*CH + c makes the final attention write
contiguous per partition.  Pad slots (>= M) carry a host-built vector w with
w @ a_src = -30000 so they vanish under exp() with no mask read or multiply.
The per-batch tgt_score is added as a [128,1] per-partition DVE scalar
operand (2 KB) instead of a broadcast [128, W] grid (800 KB).  Attention is
written back as fp16 scaled by 8192 (halves the output DMA; the host divides
the scale back out in fp32), with all 4 batches merged into ONE [p, b, c]
write per body: 1576 B descriptors (under-512 B descriptors pay a 2x DMA
latency penalty, which a per-batch fp16 write would hit) and a single
dma_start, which also moves the po pool-ring reuse a full body away from the
~0.9 us DMA-completion semaphore (measured ~1 us vs two per-pair writes).

Measured per-body attribution (hw-loop steady state, 8 cores): x DMA stream
36.6 us (349 GB/s, at the ~360 GB/s per-core bus limit), PE score pass 17.4
us, full body ~39.3 us -- DMA and PE SBUF traffic partially serialize on this
part, so the body floor is the DMA stream plus a ~5 us contention tax, not
max(DMA, PE).  Closed dead ends (all measured): fp8 x fails the 2e-2 gate
(4.0e-2 attention err); PSUM-resident tail intermediates are slower (PSUM
access latency + act-table switches, and the HW allows only one PSUM input
per instruction); Act-engine Lrelu is ~0.7us slower than DVE add+leaky
(act-table switch); ring-split DMA and pool-depth/tile-count variants are
flat; DoubleRow matmul perf mode is fp8-only."""

import numpy as np
from contextlib import ExitStack

import jax
from jax.sharding import Mesh, PartitionSpec
from jax.experimental.shard_map import shard_map

import concourse.bass as bass
import concourse.bass_isa as bass_isa
import concourse.bacc as bacc
import concourse.tile as tile
from concourse import mybir
from concourse.bass2jax import _bass_exec_p, install_neuronx_cc_hook

B, N, D = 32, 50000, 64
NCORES = 8
BPC = B // NCORES            # batches per core
PAIRS = BPC // 2             # batch-pairs per core
CH = 195                     # floor on 128-node chunks; actual ch adapts to M
NTILES = 2                   # xh DMA tiles per pair -- the measured optimum:
                             # 8->4 DMAs/body gained ~0.3us, but 1 tile/pair
                             # (2 DMAs/body, xb=3) regressed ~0.5us
XB = 6                       # x-tile pool depth
RING_SPLIT = False           # alternate x-tile DMAs across sync/scalar HWDGE
NEGW = -30000.0              # pad-slot score (vanishes under exp after lrelu)

F32 = mybir.dt.float32
F16 = mybir.dt.float16
F8 = mybir.dt.float8e4
F8NP = mybir.dt.np(F8)
AX = mybir.AxisListType
OP = mybir.AluOpType
ACT = mybir.ActivationFunctionType

# Mixed-precision contraction split: dims with the smallest |a_src| ship as
# fp8 (score-error contribution per dim scales with a_d^2).  The fp8 subset
# S is the largest with sum_S a_d^2 <= F8_SUMSQ; with x~N(0,1) and measured
# e4m3 coefficients the induced attention error is ~0.04*sqrt(F8_SUMSQ).
# DISABLED (F8_SUMSQ=0): measured a clear LOSS on TRN2 -- paired-accumulate
# fp8 matmuls ran 93us/body (PE stalls on per-chunk stationary dtype
# switches) and a dtype-grouped second pass still 57us vs 42us fp16-only
# (197 extra K=32 matmuls cost ~75ns each, far above the 12.5% DMA saving;
# deferred accumulation with skip_group_check also broke tail ordering).
F8_SUMSQ = 0.0
K8_MIN, K8_MAX = 4, 28

LAST_RUN = None

_CACHE = {}


def _tiles_c(ch, ntiles=NTILES):
    """Split ch chunks into ntiles chunk-aligned DMA tiles."""
    base, rem = divmod(ch, ntiles)
    return [base + 1] * rem + [base] * (ntiles - rem)


OUT16 = True                 # write attention as fp16 scaled by OUT_SCALE
OUT_SCALE = 8192.0


ACT_LRELU = False            # fuse add+leaky into one Act-engine Lrelu op
# 0: zb/pb in SBUF.  1: pb (exp output) in PSUM.  2: full-PSUM tail --
# Act-engine Lrelu(ps+tgt) -> zb PSUM -> Exp -> pb PSUM (only po in SBUF).
# HW rule NCC_IBVF027: an instruction may read at most ONE input from PSUM.
TAIL_PSUM = 0


def _build(reps=1, hw_loop=1, mode="full", ch=CH, ntiles=NTILES, xb=XB,
           psb=2, out16=None, ring_split=None, act_lrelu=None,
           tail_psum=None, k8=0):
    """reps: python-unrolled kernel bodies; hw_loop > 1 additionally wraps
    them in a hardware For_i loop (total bodies = reps * hw_loop) so timing
    NEFFs can amortize the ~100 ms (+/- tens of ms) axon dispatch jitter over
    hundreds of bodies without exploding the instruction count.

    mode: diagnostic bodies for attributing HW time -- "full" (the real
    kernel), "dma" (x DMA stream only), "pe" (matmuls+softmax from a static
    SBUF tile, no x DMAs), "both" (DMA stream + static-tile matmuls: no
    data dependency between the two streams)."""
    if out16 is None:
        out16 = OUT16
    if ring_split is None:
        ring_split = RING_SPLIT
    if act_lrelu is None:
        act_lrelu = ACT_LRELU
    if tail_psum is None:
        tail_psum = TAIL_PSUM
    padc = 128 * ch
    cpb = 256                # chunks per PSUM bank (512 f32 cols)
    nbank = -(-ch // cpb)    # PSUM banks per pair (1 for ch <= 256)
    if tail_psum and 2 * ch > 512:
        tail_psum = 0        # zb/pb packing needs one bank per 2*ch floats
    nb_tail = {0: 0, 1: 1, 2: 2}[tail_psum]
    assert psb * (nbank + nb_tail) <= 8, "PSUM overflow: mask too dense"

    k16 = D - k8
    nc = bacc.Bacc(trn_type="TRN2", enable_partition_id=False,
                   num_devices=NCORES)
    xs = nc.dram_tensor("xs", [PAIRS, 2 * k16, padc], F16,
                        kind="ExternalInput").ap()
    amov_d = nc.dram_tensor("amov", [2 * k16, 2], F16,
                            kind="ExternalInput").ap()
    if k8:
        xs8 = nc.dram_tensor("xs8", [PAIRS, 2 * k8, padc], F8,
                             kind="ExternalInput").ap()
        amov8_d = nc.dram_tensor("amov8", [2 * k8, 2], F8,
                                 kind="ExternalInput").ap()
    tgtv_d = nc.dram_tensor("tgtv", [128, BPC], F32,
                            kind="ExternalInput").ap()
    # [p, b, c] layout: ONE out-DMA per body with BPC*ch-element descriptors
    # (1576 B fp16 -- descriptors under 512 B pay 2x latency)
    attn = nc.dram_tensor("attn", [128, BPC, ch], F16 if out16 else F32,
                          kind="ExternalOutput").ap()

    # clamp tile size so xb bufs fit SBUF (~180 KB/partition for xpool)
    ntiles = max(ntiles, -(-xb * 256 * ch // 180000))
    tiles_c = _tiles_c(ch, ntiles)
    tiles_c8 = _tiles_c(ch, 2)   # coarser tiling for the small fp8 stream

    with tile.TileContext(nc) as tc, ExitStack() as ctx:
        singles = ctx.enter_context(tc.tile_pool(name="singles", bufs=1))
        xpool = ctx.enter_context(tc.tile_pool(name="xpool", bufs=xb))
        epool = ctx.enter_context(tc.tile_pool(name="epool", bufs=2))
        stat = ctx.enter_context(tc.tile_pool(name="stat", bufs=8))
        psco = ctx.enter_context(tc.tile_pool(name="psco", bufs=psb,
                                              space="PSUM"))

        amov_sb = singles.tile([2 * k16, 2], F16)
        with tc.high_priority():
            nc.sync.dma_start(out=amov_sb, in_=amov_d)
        if k8:
            amov8_sb = singles.tile([2 * k8, 2], F8)
            with tc.high_priority():
                nc.sync.dma_start(out=amov8_sb, in_=amov8_d)
        tgtv_sb = singles.tile([128, BPC], F32)
        nc.scalar.dma_start(out=tgtv_sb, in_=tgtv_d)
        state = {"first_tile": True}

        def body():
            for _ in range(reps):
                _one_rep()

        xstat = xstat8 = None
        if mode in ("pe", "both"):
            xstat = singles.tile([2 * k16, tiles_c[0] * 128], F16)
            nc.vector.memset(xstat, 0.0)
            if k8:
                xstat8 = singles.tile([2 * k8, tiles_c[0] * 128], F8)
                nc.vector.memset(xstat8, 0.0)

        def _one_rep():
            po = None
            if mode != "dma":
                po = epool.tile([128, BPC * ch], F16 if out16 else F32,
                                tag="po", name="po")
            for j in range(PAIRS):
                # --- load xh tiles and run matmuls per 128-node chunk ---
                xts, xts8 = [], []
                c0 = 0
                for t, tcn in enumerate(tiles_c):
                    f0, f1 = c0 * 128, (c0 + tcn) * 128
                    if mode == "pe":
                        c0 += tcn
                        continue
                    xt = xpool.tile([2 * k16, f1 - f0], F16, name="xt")
                    if state["first_tile"]:
                        # quarter the very first DMA so PE starts ~4x earlier
                        state["first_tile"] = False
                        q = (f1 - f0) // 4
                        with tc.high_priority():
                            for i in range(4):
                                nc.sync.dma_start(
                                    out=xt[:, i * q:(i + 1) * q],
                                    in_=xs[j, :, f0 + i * q:f0 + (i + 1) * q])
                    else:
                        eng = nc.scalar if (ring_split and t % 2) else nc.sync
                        eng.dma_start(out=xt, in_=xs[j, :, f0:f1])
                    xts.append((c0, xt))
                    c0 += tcn
                if k8 and mode != "pe":
                    c0 = 0
                    for tcn in tiles_c8:
                        f0, f1 = c0 * 128, (c0 + tcn) * 128
                        xt8 = xpool.tile([2 * k8, f1 - f0], F8, name="xt8",
                                         tag="xt8")
                        nc.sync.dma_start(out=xt8, in_=xs8[j, :, f0:f1])
                        xts8.append((c0, xt8))
                        c0 += tcn
                if mode == "dma":
                    continue

                ps = [psco.tile([128, 512], F32, tag=f"sc{k}",
                                name=f"ps{k}")
                      for k in range(nbank)]
                for c in range(ch):
                    if mode in ("pe", "both"):
                        lhsT = xstat[:, (c % tiles_c[0]) * 128:
                                     (c % tiles_c[0]) * 128 + 128]
                        lhsT8 = (xstat8[:, (c % tiles_c[0]) * 128:
                                        (c % tiles_c[0]) * 128 + 128]
                                 if k8 else None)
                    else:
                        tc0, xt = next((t0, x) for t0, x in reversed(xts)
                                       if t0 <= c)
                        lhsT = xt[:, (c - tc0) * 128:(c - tc0) * 128 + 128]
                        if k8:
                            tc8, xt8 = next((t0, x) for t0, x in
                                            reversed(xts8) if t0 <= c)
                            lhsT8 = xt8[:, (c - tc8) * 128:
                                        (c - tc8) * 128 + 128]
                    bk, cb = divmod(c, cpb)
                    out_ps = ps[bk][:, 2 * cb:2 * cb + 2]
                    if k8:
                        # fp16 bulk now; fp8 accumulation pass after the
                        # whole pair (one stationary-dtype switch, not 394)
                        nc.tensor.matmul(out_ps, lhsT, amov_sb,
                                         start=True, stop=False,
                                         skip_group_check=True)
                    else:
                        nc.tensor.matmul(out_ps, lhsT, amov_sb,
                                         start=True, stop=True)
                if k8:
                    for c in range(ch):
                        if mode in ("pe", "both"):
                            lhsT8 = xstat8[:, (c % tiles_c[0]) * 128:
                                           (c % tiles_c[0]) * 128 + 128]
                        else:
                            tc8, xt8 = next((t0, x) for t0, x in
                                            reversed(xts8) if t0 <= c)
                            lhsT8 = xt8[:, (c - tc8) * 128:
                                        (c - tc8) * 128 + 128]
                        bk, cb = divmod(c, cpb)
                        nc.tensor.matmul(ps[bk][:, 2 * cb:2 * cb + 2],
                                         lhsT8, amov8_sb,
                                         start=False, stop=True,
                                         skip_group_check=True)

                # --- softmax tail, straight out of PSUM ---
                # z = leaky_relu(scores + tgt, 0.01).  Scores are O(10): exp
                # cannot overflow fp32, so no max-subtraction.  Pad slots
                # carry score -30000 and vanish under exp.
                if tail_psum == 2:
                    # both parities packed into one PSUM bank per stage so
                    # tail intermediates never touch SBUF bandwidth
                    zt = psco.tile([128, 2 * ch], F32, tag="zt", name="zt")
                if tail_psum:
                    pt = psco.tile([128, 2 * ch], F32, tag="pt", name="pt")
                for bi in range(2):
                    b = 2 * j + bi
                    if tail_psum == 2:
                        zb = zt[:, bi * ch:(bi + 1) * ch]
                    else:
                        zb = epool.tile([128, ch], F32, tag=f"zb{bi}",
                                        name="zb")
                    if act_lrelu or tail_psum == 2:
                        # zb = lrelu(ps + tgt, 0.01) in one Act-engine op;
                        # single PSUM input per instruction (NCC_IBVF027)
                        for k in range(nbank):
                            c1 = min(ch, (k + 1) * cpb)
                            nc.scalar.activation(
                                zb[:, k * cpb:c1],
                                ps[k][:, bi:2 * (c1 - k * cpb):2],
                                ACT.Lrelu, bias=tgtv_sb[:, b:b + 1],
                                scale=1.0, alpha=0.01)
                    else:
                        for k in range(nbank):
                            c1 = min(ch, (k + 1) * cpb)
                            nc.vector.tensor_scalar_add(
                                zb[:, k * cpb:c1],
                                ps[k][:, bi:2 * (c1 - k * cpb):2],
                                tgtv_sb[:, b:b + 1])
                        nc.vector.scalar_tensor_tensor(zb, zb, 0.01, zb,
                                                       op0=OP.mult,
                                                       op1=OP.max)
                    # exp with per-partition row sums; global sum + broadcast
                    # in ONE idle-GPSIMD op (daisy-chain all-reduce).
                    if tail_psum:
                        pb = pt[:, bi * ch:(bi + 1) * ch]
                    else:
                        pb = epool.tile([128, ch], F32, tag=f"pb{bi}",
                                        name="pb")
                    srow = stat.tile([128, 1], F32)
                    nc.scalar.activation(pb, zb, ACT.Exp,
                                         bias=0.0, scale=1.0, accum_out=srow)
                    gsum = stat.tile([128, 1], F32, tag="gsum")
                    nc.gpsimd.partition_all_reduce(gsum, srow, 128,
                                                   bass_isa.ReduceOp.add)
                    rec = stat.tile([128, 1], F32, tag="rec")
                    nc.vector.reciprocal(rec, gsum)
                    pov = po[:, b * ch:(b + 1) * ch]
                    if out16:
                        # x OUT_SCALE keeps fp16 out of the subnormal range
                        # (attention ~ 1/25000); host divides it back out.
                        nc.vector.tensor_scalar(pov, pb, rec, OUT_SCALE,
                                                op0=OP.mult, op1=OP.mult)
                    else:
                        nc.vector.tensor_scalar_mul(pov, pb, rec)
            # one write per body: [p, (b c)] rows are contiguous in DRAM
            if mode != "dma":
                nc.scalar.dma_start(
                    out=attn.rearrange("p b c -> p (b c)"), in_=po)

        if hw_loop > 1:
            with tc.For_i(0, hw_loop):
                body()
        else:
            body()
    nc.compile()
    return nc


def _host_prep(x, a, node_index, adj_mask):
    x = np.asarray(x, dtype=np.float32)
    a = np.asarray(a, dtype=np.float32).reshape(2 * D)
    adj = np.asarray(adj_mask)
    idx = int(node_index)
    a_tgt, a_src = a[:D], a[D:]

    tgt = (x[:, idx, :] @ a_tgt).astype(np.float32)          # [B]

    keep = np.flatnonzero(adj > 0)                           # [M] node ids
    m = len(keep)
    assert m > 0, "all-masked adjacency not supported"
    ch = max(CH, -(-m // 128))                               # capacity chunks
    padc = 128 * ch

    # fp8 subset: smallest-|a| dims with cumulative a^2 under the budget
    order = np.argsort(np.abs(a_src), kind="stable")
    cum = np.cumsum(a_src[order] ** 2)
    k8 = int(np.searchsorted(cum, F8_SUMSQ, side="right"))
    k8 = min(k8, K8_MAX)
    if k8 < K8_MIN:
        k8 = 0
    dims8, dims16 = order[:k8], np.sort(order[k8:])
    k16 = D - k8
    a16 = a_src[dims16]

    # col = c*128 + p  <->  compact slot k = p*ch + c
    kk = (np.arange(ch)[:, None] + np.arange(128)[None, :] * ch).ravel()
    valid = kk < m
    colidx = keep[np.minimum(kk, m - 1)]

    # xh16[pair, bi*k16+d, col] = x[2*pair+bi, node(col), dims16[d]]  (fp16)
    xt16 = np.ascontiguousarray(x.transpose(0, 2, 1), dtype=np.float16)
    xh16 = xt16[:, dims16][:, :, colidx]                     # [B, k16, padc]
    # pad slots carry the full -30000 score via the fp16 rows alone
    w = (a16 * (NEGW / max(float(a16 @ a16), 1e-12))).astype(np.float16)
    xh16[:, :, ~valid] = w[None, :, None]
    xh16 = np.ascontiguousarray(xh16).reshape(B // 2, 2 * k16, padc)

    amov16 = np.zeros((2 * k16, 2), np.float16)
    amov16[0:k16, 0] = a16
    amov16[k16:, 1] = a16

    if k8:
        xh8 = xt16[:, dims8][:, :, colidx].astype(F8NP)      # [B, k8, padc]
        xh8[:, :, ~valid] = F8NP(0.0)
        xh8 = np.ascontiguousarray(xh8).reshape(B // 2, 2 * k8, padc)
        amov8 = np.zeros((2 * k8, 2), F8NP)
        amov8[0:k8, 0] = a_src[dims8].astype(F8NP)
        amov8[k8:, 1] = a_src[dims8].astype(F8NP)
    else:
        xh8, amov8 = None, None

    return xh16, xh8, amov16, amov8, tgt, keep, ch, k8


def _in_maps(xh16, xh8, amov16, amov8, tgt, keep, ch, k8):
    maps = []
    for c in range(NCORES):
        tgtv = np.ascontiguousarray(np.broadcast_to(
            tgt[c * BPC:(c + 1) * BPC][None, :], (128, BPC)).astype(
                np.float32))
        mp = {
            "xs": xh16[c * PAIRS:(c + 1) * PAIRS],
            "amov": amov16,
            "tgtv": tgtv,
        }
        if k8:
            mp["xs8"] = xh8[c * PAIRS:(c + 1) * PAIRS]
            mp["amov8"] = amov8
        maps.append(mp)
    return maps


def _runner(ch=CH, k8=0):
    """Build the Bass program once and wrap its NEFF custom call in a jitted
    shard_map over the 8 cores."""
    key = ("runner", ch, k8)
    if key in _CACHE:
        return _CACHE[key]
    install_neuronx_cc_hook()
    nc = _CACHE.setdefault(("nc", ch, k8), _build(ch=ch, k8=k8))
    in_names, out_names, out_avals, zero_shapes = [], [], [], []
    for alloc in nc.m.functions[0].allocations:
        if not isinstance(alloc, mybir.MemoryLocationSet):
            continue
        name = alloc.memorylocations[0].name
        if alloc.kind == "ExternalInput":
            in_names.append(name)
        elif alloc.kind == "ExternalOutput":
            out_names.append(name)
            shape = tuple(alloc.tensor_shape)
            dtype = mybir.dt.np(alloc.dtype)
            out_avals.append(jax.core.ShapedArray(shape, dtype))
            zero_shapes.append((shape, dtype))

    def _body(*args):
        return tuple(_bass_exec_p.bind(
            *args,
            out_avals=tuple(out_avals),
            in_names=tuple(in_names + out_names),
            out_names=tuple(out_names),
            lowering_input_output_aliases=(),
            sim_require_finite=True,
            sim_require_nnan=True,
            nc=nc,
        ))

    mesh = Mesh(np.asarray(jax.devices()[:NCORES]), ("core",))
    nin = len(in_names) + len(out_names)
    sharded = jax.jit(shard_map(
        _body, mesh=mesh,
        in_specs=(PartitionSpec("core"),) * nin,
        out_specs=(PartitionSpec("core"),) * len(out_names),
        check_rep=False))
    _CACHE[key] = (sharded, in_names, out_names, zero_shapes)
    return _CACHE[key]


def kernel(x, a, node_index, adj_mask):
    global LAST_RUN
    prep = _host_prep(x, a, node_index, adj_mask)
    keep, ch, k8 = prep[5], prep[6], prep[7]
    m = len(keep)
    maps = _in_maps(*prep)
    sharded, in_names, out_names, zero_shapes = _runner(ch=ch, k8=k8)
    # concat of the 8 per-core xs/xs8 shards is exactly the full arrays
    full = {"xs": prep[0], "xs8": prep[1]}
    ins = [full[nm] if nm in full else
           np.concatenate([mp[nm] for mp in maps], axis=0)
           for nm in in_names]
    zeros = [np.zeros((NCORES * s[0], *s[1:]), d) for s, d in zero_shapes]
    outs = sharded(*ins, *zeros)
    LAST_RUN = outs
    # [NCORES*128, BPC, ch]: partition-major per core, batch, chunk
    attn_c = np.asarray(outs[out_names.index("attn")])
    flat = attn_c.reshape(NCORES, 128, BPC, ch).transpose(0, 2, 1, 3) \
        .reshape(B, 128 * ch)                                 # [b, p*ch+c]
    full = np.zeros((B, N), np.float32)
    if attn_c.dtype == np.float16:
        full[:, keep] = flat[:, :m].astype(np.float32) * (1.0 / OUT_SCALE)
    else:
        full[:, keep] = flat[:, :m]
    return full
